# revision 3
# baseline (speedup 1.0000x reference)
"""Multi-head causal self-attention (B=2, T=2048, C=768, H=12, D=64) on 8
Trainium2 NeuronCores.

Sharding: 24 (batch, head) units -> 3 heads per core; cores 0-3 take batch 0,
cores 4-7 take batch 1. Each core computes q/k/v projections for its 3 heads,
flash-style causal attention fully on-chip (no T x T tensor ever touches HBM),
and a partial output projection with its 192-row slice of Wproj. The host sums
the 4 partial projections per batch.

Device layout choices (all matmuls in float32r: full PE rate, ~1e-4 rel err):
  - x is pre-transposed on host to xT [C, T] and augmented with a ones row,
    so QKV contractions run with C on SBUF partitions.
  - q^T/k^T are produced in [64, T]-per-head layout (heads 0/1 packed in one
    [128, T] tile at base partitions 0/64 -> paired S^T matmuls can run
    concurrently on disjoint PE row-groups).
  - S^T = K^T.T @ Q^T is computed transposed [tk, tq] so exp(S^T) feeds the
    P.T @ V matmul directly - no on-chip transposes anywhere.
  - V is augmented with a ones column per head (generated by the matmul via
    the xT ones row), so the PV accumulation yields the softmax denominator
    as row 64 of the O^T psum for free.
  - Causal masking: matmuls restrict tq columns to >= the tk block start;
    the 128x128 diagonal sub-block gets a strictly-lower-triangular -1e30
    additive mask before exp.
"""

import os
import sys

sys.path.insert(0, "/opt/trn_rl_repo")

import numpy as np

import concourse.bass as bass
import concourse.tile as tile
from concourse import bacc, mybir
from concourse import bass_utils
from concourse.masks import make_lower_triangular

B, T, C = 2, 2048, 768
H, D = 12, 64
N_CORES = 8
H_LOC = 3           # heads per core
DL = H_LOC * D      # 192 local head dims
TQ = 512            # tq chunk (psum bank width)
TB = 128            # tk block
NCH = T // TQ       # 4 chunks
NBL = TQ // TB      # 4 blocks per chunk
NKT = C // 128      # 6 contraction k-tiles

f32 = mybir.dt.float32
f32r = mybir.dt.float32r
EXP = mybir.ActivationFunctionType.Exp

LAST_RESULT = None  # test harness reads exec_time_ns from here


def _build_program(use_qk_bias: bool):
    from contextlib import ExitStack

    nc = bacc.Bacc("TRN2", target_bir_lowering=False, debug=False,
                   num_devices=N_CORES)

    xt_d = nc.dram_tensor("xt", [C + 1, T], f32, kind="ExternalInput").ap()
    wqk_d = nc.dram_tensor("wqk", [C + 1, 2 * DL], f32, kind="ExternalInput").ap()
    wv_d = nc.dram_tensor("wv", [C + 1, 256], f32, kind="ExternalInput").ap()
    wp_d = nc.dram_tensor("wp", [DL, C], f32, kind="ExternalInput").ap()
    out_d = nc.dram_tensor("outT", [C, T], f32, kind="ExternalOutput").ap()

    with tile.TileContext(nc) as tc, ExitStack() as ctx:
        cpool = ctx.enter_context(tc.tile_pool(name="const", bufs=1))
        wpool = ctx.enter_context(tc.tile_pool(name="w", bufs=1))
        xpool = ctx.enter_context(tc.tile_pool(name="x", bufs=1))
        qkpool = ctx.enter_context(tc.tile_pool(name="qk", bufs=1))

        tri = cpool.tile([128, 128], f32)
        make_lower_triangular(nc, tri[:], val=-1e30, diag=False)

        # --- weights + x loads ---
        xt = xpool.tile([128, NKT, T], f32r)
        nc.sync.dma_start(xt[:], xt_d[0:C].rearrange("(n p) m -> p n m", p=128).bitcast(f32r))
        xt1 = xpool.tile([1, T], f32r)
        nc.sync.dma_start(xt1[:], xt_d[C : C + 1].bitcast(f32r))

        wqk = wpool.tile([128, NKT, 2 * DL], f32r)
        nc.sync.dma_start(wqk[:], wqk_d[0:C].rearrange("(n p) m -> p n m", p=128).bitcast(f32r))
        wqk1 = wpool.tile([1, 2 * DL], f32r)
        nc.sync.dma_start(wqk1[:], wqk_d[C : C + 1].bitcast(f32r))
        wv = wpool.tile([128, NKT, 256], f32r)
        nc.sync.dma_start(wv[:], wv_d[0:C].rearrange("(n p) m -> p n m", p=128).bitcast(f32r))
        wv1 = wpool.tile([1, 256], f32r)
        nc.sync.dma_start(wv1[:], wv_d[C : C + 1].bitcast(f32r))
        wp = wpool.tile([128, C], f32r)
        nc.sync.dma_start(wp[:], wp_d[0:128].bitcast(f32r))
        wp2 = wpool.tile([64, C], f32r)
        nc.sync.dma_start(wp2[:], wp_d[128:DL].bitcast(f32r))

        # --- QKV phase ---
        # M-tiles of the packed wqk: [q0|q1], [k0|k1], [q2|k2]
        qT01 = qkpool.tile([128, T], f32r, tag="qT01")
        kT01 = qkpool.tile([128, T], f32r, tag="kT01")
        qT2 = qkpool.tile([64, T], f32r, tag="qT2")
        kT2 = qkpool.tile([64, T], f32r, tag="kT2")
        v_sb = qkpool.tile([128, T // TB, 3 * (D + 1)], f32r, tag="v")

        with tc.tile_pool(name="qkv_ps", bufs=2, space="PSUM") as qkv_ps, \
             tc.tile_pool(name="v_ps", bufs=2, space="PSUM") as v_ps:
            for m in range(3):
                for t in range(NCH):
                    ps = qkv_ps.tile([128, TQ], f32)
                    for j in range(NKT):
                        nc.tensor.matmul(
                            ps[:],
                            wqk[:, j, 128 * m : 128 * (m + 1)],
                            xt[:, j, TQ * t : TQ * (t + 1)],
                            start=(j == 0),
                            stop=(j == NKT - 1 and not use_qk_bias),
                        )
                    if use_qk_bias:
                        nc.tensor.matmul(
                            ps[:],
                            wqk1[:, 128 * m : 128 * (m + 1)],
                            xt1[:, TQ * t : TQ * (t + 1)],
                            start=False, stop=True,
                        )
                    sl = (slice(None), slice(TQ * t, TQ * (t + 1)))
                    if m == 0:
                        nc.vector.tensor_copy(qT01[sl], ps[:])
                    elif m == 1:
                        nc.vector.tensor_copy(kT01[sl], ps[:])
                    else:
                        nc.vector.tensor_copy(qT2[:, TQ * t : TQ * (t + 1)], ps[0:64, :])
                        nc.vector.tensor_copy(kT2[:, TQ * t : TQ * (t + 1)], ps[64:128, :])
            # v in [t, d] layout; wv columns interleave [v_h | ones] per head,
            # the ones col + bias row ride on the xt ones row (k-tile 7).
            for t in range(T // TB):
                psv = v_ps.tile([128, 256], f32)
                for j in range(NKT):
                    nc.tensor.matmul(
                        psv[:],
                        xt[:, j, TB * t : TB * (t + 1)],
                        wv[:, j, :],
                        start=(j == 0), stop=False,
                    )
                nc.tensor.matmul(
                    psv[:],
                    xt1[:, TB * t : TB * (t + 1)],
                    wv1[:],
                    start=False, stop=True,
                )
                nc.vector.tensor_copy(v_sb[:, t, :], psv[:, 0 : 3 * (D + 1)])

        # --- attention + projection ---
        prhs0 = qkpool.tile([128, T], f32r, tag="prhs0")   # heads 0,1 normalized O^T
        prhs1 = qkpool.tile([64, T], f32r, tag="prhs1")    # head 2

        qT = [qT01[0:64, :], qT01[64:128, :], qT2[:, :]]
        kT = [kT01[0:64, :], kT01[64:128, :], kT2[:, :]]

        with tc.tile_pool(name="s_ps", bufs=3, space="PSUM") as s_ps, \
             tc.tile_pool(name="po_ps", bufs=2, space="PSUM") as po_ps, \
             tc.tile_pool(name="pb_ps", bufs=2, space="PSUM") as pb_ps, \
             tc.tile_pool(name="pt_p", bufs=4) as pt_p, \
             tc.tile_pool(name="nrm", bufs=2) as nrm, \
             tc.tile_pool(name="outp", bufs=3) as outp:
            for i in range(NCH):
                nblk = NBL * (i + 1)
                for h in range(H_LOC):
                    po = po_ps.tile([D + 1, TQ], f32, tag="po")
                    for Bq in range(nblk):
                        j = Bq - NBL * i
                        c0 = 0 if j < 0 else min(TB * j, TQ - 256)
                        ps = s_ps.tile([128, TQ], f32, tag="s")
                        nc.tensor.matmul(
                            ps[:, c0:TQ],
                            kT[h][:, TB * Bq : TB * (Bq + 1)],
                            qT[h][:, TQ * i + c0 : TQ * (i + 1)],
                            start=True, stop=True,
                        )
                        if j >= 0:
                            if TB * j > c0:
                                nc.vector.tensor_scalar_add(
                                    ps[:, c0 : TB * j], ps[:, c0 : TB * j], -1e30)
                            nc.vector.tensor_add(
                                ps[:, TB * j : TB * (j + 1)],
                                ps[:, TB * j : TB * (j + 1)],
                                tri[:],
                            )
                        pt = pt_p.tile([128, TQ], f32r, tag="pt")
                        nc.scalar.activation(pt[:, c0:TQ], ps[:, c0:TQ], EXP)
                        nc.tensor.matmul(
                            po[:, c0:TQ],
                            v_sb[:, Bq, (D + 1) * h : (D + 1) * (h + 1)],
                            pt[:, c0:TQ],
                            start=(Bq == 0), stop=(Bq == nblk - 1),
                        )
                    # normalize: row D of po is the softmax denominator
                    d_sb = nrm.tile([1, TQ], f32r, tag="d")
                    with nc.allow_low_precision(reason="fp32r feeds matmul"):
                        nc.vector.reciprocal(d_sb[:], po[D : D + 1, :])
                    pb = pb_ps.tile([D, TQ], f32, tag="pb")
                    nc.tensor.matmul(pb[:], xt1[0:1, 0:D], d_sb[:], start=True, stop=True)
                    o_sb = nrm.tile([D, TQ], f32, tag="o")
                    nc.vector.tensor_copy(o_sb[:], po[0:D, :])
                    dst = (prhs0[64 * h : 64 * (h + 1), TQ * i : TQ * (i + 1)]
                           if h < 2 else prhs1[:, TQ * i : TQ * (i + 1)])
                    nc.vector.tensor_mul(dst, o_sb[:], pb[:])
                # projection for this chunk
                for n in range(C // 128):
                    pp = pb_ps.tile([128, TQ], f32, tag="pb")
                    nc.tensor.matmul(pp[:], wp[:, 128 * n : 128 * (n + 1)],
                                     prhs0[:, TQ * i : TQ * (i + 1)],
                                     start=True, stop=False)
                    nc.tensor.matmul(pp[:], wp2[:, 128 * n : 128 * (n + 1)],
                                     prhs1[:, TQ * i : TQ * (i + 1)],
                                     start=False, stop=True)
                    osb = outp.tile([128, TQ], f32, tag="out")
                    nc.vector.tensor_copy(osb[:], pp[:])
                    nc.sync.dma_start(
                        out_d[128 * n : 128 * (n + 1), TQ * i : TQ * (i + 1)], osb[:])

    nc.compile()
    return nc


_PROG_CACHE = {}


def kernel(x, Wqkv, bqkv, Wproj, bproj):
    global LAST_RESULT
    x = np.asarray(x, dtype=np.float32)
    Wqkv = np.asarray(Wqkv, dtype=np.float32)
    bqkv = np.asarray(bqkv, dtype=np.float32)
    Wproj = np.asarray(Wproj, dtype=np.float32)
    bproj = np.asarray(bproj, dtype=np.float32)

    Wq, Wk, Wv = Wqkv[:, 0:C], Wqkv[:, C : 2 * C], Wqkv[:, 2 * C : 3 * C]
    bq, bk, bv = bqkv[0:C], bqkv[C : 2 * C], bqkv[2 * C : 3 * C]
    scale = 1.0 / np.sqrt(D)

    use_qk_bias = bool(np.any(bq) or np.any(bk))
    key = use_qk_bias
    if key not in _PROG_CACHE:
        _PROG_CACHE[key] = _build_program(use_qk_bias)
    nc = _PROG_CACHE[key]

    in_maps = []
    for c in range(N_CORES):
        b = c // (N_CORES // B)
        g = c % (N_CORES // B)
        hs = slice(DL * g, DL * (g + 1))       # this core's head-dim rows/cols

        xt = np.empty((C + 1, T), np.float32)
        xt[0:C] = x[b].T
        xt[C] = 1.0

        wq_loc = Wq[:, hs] * scale             # fold 1/sqrt(D) into q
        bq_loc = bq[hs] * scale
        wk_loc, bk_loc = Wk[:, hs], bk[hs]
        wv_loc, bv_loc = Wv[:, hs], bv[hs]

        wqk = np.empty((C + 1, 2 * DL), np.float32)
        wqk[0:C, 0:128] = wq_loc[:, 0:128]
        wqk[C, 0:128] = bq_loc[0:128]
        wqk[0:C, 128:256] = wk_loc[:, 0:128]
        wqk[C, 128:256] = bk_loc[0:128]
        wqk[0:C, 256:320] = wq_loc[:, 128:192]
        wqk[C, 256:320] = bq_loc[128:192]
        wqk[0:C, 320:384] = wk_loc[:, 128:192]
        wqk[C, 320:384] = bk_loc[128:192]

        wv_pad = np.zeros((C + 1, 256), np.float32)
        for h in range(H_LOC):
            c0 = (D + 1) * h
            wv_pad[0:C, c0 : c0 + D] = wv_loc[:, D * h : D * (h + 1)]
            wv_pad[C, c0 : c0 + D] = bv_loc[D * h : D * (h + 1)]
            wv_pad[C, c0 + D] = 1.0            # ones column -> softmax denom

        wp = np.ascontiguousarray(Wproj[DL * g : DL * (g + 1), :])

        in_maps.append({"xt": xt, "wqk": wqk, "wv": wv_pad, "wp": wp})

    res = bass_utils.run_bass_kernel_spmd(nc, in_maps, core_ids=list(range(N_CORES)))
    LAST_RESULT = res

    out = np.zeros((B, T, C), np.float32)
    for c in range(N_CORES):
        b = c // (N_CORES // B)
        out[b] += res.results[c]["outT"].T
    return out + bproj


if __name__ == "__main__":
    rng = np.random.default_rng(0)
    s = 1.0 / np.sqrt(C)
    ins = {
        "x": rng.standard_normal((B, T, C), dtype=np.float32),
        "Wqkv": rng.standard_normal((C, 3 * C), dtype=np.float32) * s,
        "bqkv": np.zeros(3 * C, np.float32),
        "Wproj": rng.standard_normal((C, C), dtype=np.float32) * s,
        "bproj": np.zeros(C, np.float32),
    }
    out = kernel(**ins)
    print("out", out.shape, out.dtype, float(np.abs(out).max()))


# revision 4
# speedup vs baseline: 1.0586x; 1.0586x over previous
"""Multi-head causal self-attention (B=2, T=2048, C=768, H=12, D=64) on 8
Trainium2 NeuronCores.

Sharding: 24 (batch, head) units -> 3 heads per core; cores 0-3 take batch 0,
cores 4-7 take batch 1. Each core computes q/k/v projections for its 3 heads,
flash-style causal attention fully on-chip (no T x T tensor ever touches HBM),
and a partial output projection with its 192-row slice of Wproj. The host sums
the 4 partial projections per batch.

Device layout choices (all matmuls in float32r: full PE rate, ~1e-4 rel err):
  - x is pre-transposed on host to xT [C, T] and augmented with a ones row,
    so QKV contractions run with C on SBUF partitions.
  - q^T/k^T are produced in [64, T]-per-head layout (heads 0/1 packed in one
    [128, T] tile at base partitions 0/64 -> paired S^T matmuls can run
    concurrently on disjoint PE row-groups).
  - S^T = K^T.T @ Q^T is computed transposed [tk, tq] so exp(S^T) feeds the
    P.T @ V matmul directly - no on-chip transposes anywhere.
  - V is augmented with a ones column per head (generated by the matmul via
    the xT ones row), so the PV accumulation yields the softmax denominator
    as row 64 of the O^T psum for free.
  - Causal masking: matmuls restrict tq columns to >= the tk block start;
    the 128x128 diagonal sub-block gets a strictly-lower-triangular -1e30
    additive mask before exp.
"""

import os
import sys

sys.path.insert(0, "/opt/trn_rl_repo")

import numpy as np

import concourse.bass as bass
import concourse.tile as tile
from concourse import bacc, mybir
from concourse import bass_utils
from concourse.masks import make_lower_triangular

B, T, C = 2, 2048, 768
H, D = 12, 64
N_CORES = 8
H_LOC = 3           # heads per core
DL = H_LOC * D      # 192 local head dims
TQ = 512            # tq chunk (psum bank width)
TB = 128            # tk block
NCH = T // TQ       # 4 chunks
NBL = TQ // TB      # 4 blocks per chunk
NKT = C // 128      # 6 contraction k-tiles

f32 = mybir.dt.float32
f32r = mybir.dt.float32r
EXP = mybir.ActivationFunctionType.Exp

LAST_RESULT = None  # test harness reads exec_time_ns from here


def _build_program(use_qk_bias: bool):
    from contextlib import ExitStack

    nc = bacc.Bacc("TRN2", target_bir_lowering=False, debug=False,
                   num_devices=N_CORES)

    xt_d = nc.dram_tensor("xt", [C + 1, T], f32, kind="ExternalInput").ap()
    wqk_d = nc.dram_tensor("wqk", [C + 1, 2 * DL], f32, kind="ExternalInput").ap()
    wv_d = nc.dram_tensor("wv", [C + 1, 256], f32, kind="ExternalInput").ap()
    wp_d = nc.dram_tensor("wp", [DL, C], f32, kind="ExternalInput").ap()
    out_d = nc.dram_tensor("outT", [C, T], f32, kind="ExternalOutput").ap()

    with tile.TileContext(nc) as tc, ExitStack() as ctx:
        cpool = ctx.enter_context(tc.tile_pool(name="const", bufs=1))
        wpool = ctx.enter_context(tc.tile_pool(name="w", bufs=1))
        xpool = ctx.enter_context(tc.tile_pool(name="x", bufs=1))
        qkpool = ctx.enter_context(tc.tile_pool(name="qk", bufs=1))

        tri = cpool.tile([128, 128], f32)
        make_lower_triangular(nc, tri[:], val=-1e30, diag=False)

        # --- weights + x loads ---
        xt = xpool.tile([128, NKT, T], f32r)
        nc.sync.dma_start(xt[:], xt_d[0:C].rearrange("(n p) m -> p n m", p=128).bitcast(f32r))
        xt1 = xpool.tile([1, T], f32r)
        nc.sync.dma_start(xt1[:], xt_d[C : C + 1].bitcast(f32r))

        wqk = wpool.tile([128, NKT, 2 * DL], f32r)
        nc.sync.dma_start(wqk[:], wqk_d[0:C].rearrange("(n p) m -> p n m", p=128).bitcast(f32r))
        wqk1 = wpool.tile([1, 2 * DL], f32r)
        nc.sync.dma_start(wqk1[:], wqk_d[C : C + 1].bitcast(f32r))
        wv = wpool.tile([128, NKT, 256], f32r)
        nc.sync.dma_start(wv[:], wv_d[0:C].rearrange("(n p) m -> p n m", p=128).bitcast(f32r))
        wv1 = wpool.tile([1, 256], f32r)
        nc.sync.dma_start(wv1[:], wv_d[C : C + 1].bitcast(f32r))
        wp = wpool.tile([128, C], f32r)
        nc.sync.dma_start(wp[:], wp_d[0:128].bitcast(f32r))
        wp2 = wpool.tile([64, C], f32r)
        nc.sync.dma_start(wp2[:], wp_d[128:DL].bitcast(f32r))

        # --- QKV phase ---
        # M-tiles of the packed wqk: [q0|q1], [k0|k1], [q2|k2]
        qT01 = qkpool.tile([128, T], f32r, tag="qT01")
        kT01 = qkpool.tile([128, T], f32r, tag="kT01")
        qT2 = qkpool.tile([64, T], f32r, tag="qT2")
        kT2 = qkpool.tile([64, T], f32r, tag="kT2")
        v_sb = qkpool.tile([128, T // TB, 3 * (D + 1)], f32r, tag="v")

        with tc.tile_pool(name="qkv_ps", bufs=2, space="PSUM") as qkv_ps, \
             tc.tile_pool(name="v_ps", bufs=2, space="PSUM") as v_ps:
            for m in range(3):
                for t in range(NCH):
                    ps = qkv_ps.tile([128, TQ], f32)
                    for j in range(NKT):
                        nc.tensor.matmul(
                            ps[:],
                            wqk[:, j, 128 * m : 128 * (m + 1)],
                            xt[:, j, TQ * t : TQ * (t + 1)],
                            start=(j == 0),
                            stop=(j == NKT - 1 and not use_qk_bias),
                        )
                    if use_qk_bias:
                        nc.tensor.matmul(
                            ps[:],
                            wqk1[:, 128 * m : 128 * (m + 1)],
                            xt1[:, TQ * t : TQ * (t + 1)],
                            start=False, stop=True,
                        )
                    sl = (slice(None), slice(TQ * t, TQ * (t + 1)))
                    if m == 0:
                        nc.vector.tensor_copy(qT01[sl], ps[:])
                    elif m == 1:
                        nc.vector.tensor_copy(kT01[sl], ps[:])
                    else:
                        nc.vector.tensor_copy(qT2[:, TQ * t : TQ * (t + 1)], ps[0:64, :])
                        nc.vector.tensor_copy(kT2[:, TQ * t : TQ * (t + 1)], ps[64:128, :])
            # v in [t, d] layout; wv columns interleave [v_h | ones] per head,
            # the ones col + bias row ride on the xt ones row (k-tile 7).
            for t in range(T // TB):
                psv = v_ps.tile([128, 256], f32)
                for j in range(NKT):
                    nc.tensor.matmul(
                        psv[:],
                        xt[:, j, TB * t : TB * (t + 1)],
                        wv[:, j, :],
                        start=(j == 0), stop=False,
                    )
                nc.tensor.matmul(
                    psv[:],
                    xt1[:, TB * t : TB * (t + 1)],
                    wv1[:],
                    start=False, stop=True,
                )
                nc.vector.tensor_copy(v_sb[:, t, :], psv[:, 0 : 3 * (D + 1)])

        # --- attention + projection ---
        prhs0 = qkpool.tile([128, T], f32r, tag="prhs0")   # heads 0,1 normalized O^T
        prhs1 = qkpool.tile([64, T], f32r, tag="prhs1")    # head 2

        qT = [qT01[0:64, :], qT01[64:128, :], qT2[:, :]]
        kT = [kT01[0:64, :], kT01[64:128, :], kT2[:, :]]

        with tc.tile_pool(name="s_ps", bufs=3, space="PSUM") as s_ps, \
             tc.tile_pool(name="po_ps", bufs=2, space="PSUM") as po_ps, \
             tc.tile_pool(name="pb_ps", bufs=2, space="PSUM") as pb_ps, \
             tc.tile_pool(name="pt_p", bufs=4) as pt_p, \
             tc.tile_pool(name="nrm", bufs=2) as nrm, \
             tc.tile_pool(name="outp", bufs=3) as outp:
            for i in range(NCH):
                nblk = NBL * (i + 1)
                for h in range(H_LOC):
                    po = po_ps.tile([D + 1, TQ], f32, tag="po")
                    for Bq in range(nblk):
                        j = Bq - NBL * i
                        c0 = 0 if j < 0 else min(TB * j, TQ - 256)
                        ps = s_ps.tile([128, TQ], f32, tag="s")
                        nc.tensor.matmul(
                            ps[:, c0:TQ],
                            kT[h][:, TB * Bq : TB * (Bq + 1)],
                            qT[h][:, TQ * i + c0 : TQ * (i + 1)],
                            start=True, stop=True,
                        )
                        if j >= 0:
                            if TB * j > c0:
                                nc.vector.tensor_scalar_add(
                                    ps[:, c0 : TB * j], ps[:, c0 : TB * j], -1e30)
                            nc.vector.tensor_add(
                                ps[:, TB * j : TB * (j + 1)],
                                ps[:, TB * j : TB * (j + 1)],
                                tri[:],
                            )
                        pt = pt_p.tile([128, TQ], f32r, tag="pt")
                        nc.scalar.activation(pt[:, c0:TQ], ps[:, c0:TQ], EXP)
                        nc.tensor.matmul(
                            po[:, c0:TQ],
                            v_sb[:, Bq, (D + 1) * h : (D + 1) * (h + 1)],
                            pt[:, c0:TQ],
                            start=(Bq == 0), stop=(Bq == nblk - 1),
                        )
                    # normalize: row D of po is the softmax denominator.
                    # Broadcast it across 64 partitions via a rank-1 matmul,
                    # take a fast approx reciprocal, multiply into O^T.
                    d_sb = nrm.tile([1, TQ], f32r, tag="d")
                    nc.vector.tensor_copy(d_sb[:], po[D : D + 1, :])
                    pb = pb_ps.tile([D, TQ], f32, tag="pb")
                    nc.tensor.matmul(pb[:], xt1[0:1, 0:D], d_sb[:], start=True, stop=True)
                    rb = nrm.tile([D, TQ], f32, tag="rb")
                    nc.vector.reciprocal_approx_fast(rb[:], pb[:])
                    dst = (prhs0[64 * h : 64 * (h + 1), TQ * i : TQ * (i + 1)]
                           if h < 2 else prhs1[:, TQ * i : TQ * (i + 1)])
                    nc.vector.tensor_mul(dst, po[0:D, :], rb[:])
                # projection for this chunk
                for n in range(C // 128):
                    pp = pb_ps.tile([128, TQ], f32, tag="pb")
                    nc.tensor.matmul(pp[:], wp[:, 128 * n : 128 * (n + 1)],
                                     prhs0[:, TQ * i : TQ * (i + 1)],
                                     start=True, stop=False)
                    nc.tensor.matmul(pp[:], wp2[:, 128 * n : 128 * (n + 1)],
                                     prhs1[:, TQ * i : TQ * (i + 1)],
                                     start=False, stop=True)
                    osb = outp.tile([128, TQ], f32, tag="out")
                    nc.vector.tensor_copy(osb[:], pp[:])
                    nc.sync.dma_start(
                        out_d[128 * n : 128 * (n + 1), TQ * i : TQ * (i + 1)], osb[:])

    nc.compile()
    return nc


_PROG_CACHE = {}


def kernel(x, Wqkv, bqkv, Wproj, bproj):
    global LAST_RESULT
    x = np.asarray(x, dtype=np.float32)
    Wqkv = np.asarray(Wqkv, dtype=np.float32)
    bqkv = np.asarray(bqkv, dtype=np.float32)
    Wproj = np.asarray(Wproj, dtype=np.float32)
    bproj = np.asarray(bproj, dtype=np.float32)

    Wq, Wk, Wv = Wqkv[:, 0:C], Wqkv[:, C : 2 * C], Wqkv[:, 2 * C : 3 * C]
    bq, bk, bv = bqkv[0:C], bqkv[C : 2 * C], bqkv[2 * C : 3 * C]
    scale = 1.0 / np.sqrt(D)

    use_qk_bias = bool(np.any(bq) or np.any(bk))
    key = use_qk_bias
    if key not in _PROG_CACHE:
        _PROG_CACHE[key] = _build_program(use_qk_bias)
    nc = _PROG_CACHE[key]

    in_maps = []
    for c in range(N_CORES):
        b = c // (N_CORES // B)
        g = c % (N_CORES // B)
        hs = slice(DL * g, DL * (g + 1))       # this core's head-dim rows/cols

        xt = np.empty((C + 1, T), np.float32)
        xt[0:C] = x[b].T
        xt[C] = 1.0

        wq_loc = Wq[:, hs] * scale             # fold 1/sqrt(D) into q
        bq_loc = bq[hs] * scale
        wk_loc, bk_loc = Wk[:, hs], bk[hs]
        wv_loc, bv_loc = Wv[:, hs], bv[hs]

        wqk = np.empty((C + 1, 2 * DL), np.float32)
        wqk[0:C, 0:128] = wq_loc[:, 0:128]
        wqk[C, 0:128] = bq_loc[0:128]
        wqk[0:C, 128:256] = wk_loc[:, 0:128]
        wqk[C, 128:256] = bk_loc[0:128]
        wqk[0:C, 256:320] = wq_loc[:, 128:192]
        wqk[C, 256:320] = bq_loc[128:192]
        wqk[0:C, 320:384] = wk_loc[:, 128:192]
        wqk[C, 320:384] = bk_loc[128:192]

        wv_pad = np.zeros((C + 1, 256), np.float32)
        for h in range(H_LOC):
            c0 = (D + 1) * h
            wv_pad[0:C, c0 : c0 + D] = wv_loc[:, D * h : D * (h + 1)]
            wv_pad[C, c0 : c0 + D] = bv_loc[D * h : D * (h + 1)]
            wv_pad[C, c0 + D] = 1.0            # ones column -> softmax denom

        wp = np.ascontiguousarray(Wproj[DL * g : DL * (g + 1), :])

        in_maps.append({"xt": xt, "wqk": wqk, "wv": wv_pad, "wp": wp})

    res = bass_utils.run_bass_kernel_spmd(nc, in_maps, core_ids=list(range(N_CORES)))
    LAST_RESULT = res

    out = np.zeros((B, T, C), np.float32)
    for c in range(N_CORES):
        b = c // (N_CORES // B)
        out[b] += res.results[c]["outT"].T
    return out + bproj


if __name__ == "__main__":
    rng = np.random.default_rng(0)
    s = 1.0 / np.sqrt(C)
    ins = {
        "x": rng.standard_normal((B, T, C), dtype=np.float32),
        "Wqkv": rng.standard_normal((C, 3 * C), dtype=np.float32) * s,
        "bqkv": np.zeros(3 * C, np.float32),
        "Wproj": rng.standard_normal((C, C), dtype=np.float32) * s,
        "bproj": np.zeros(C, np.float32),
    }
    out = kernel(**ins)
    print("out", out.shape, out.dtype, float(np.abs(out).max()))


# revision 9
# speedup vs baseline: 1.5101x; 1.4264x over previous
"""Multi-head causal self-attention (B=2, T=2048, C=768, H=12, D=64) on 8
Trainium2 NeuronCores.

Sharding: 24 (batch, head) units -> 3 heads per core; cores 0-3 take batch 0,
cores 4-7 take batch 1. Each core computes q/k/v projections for its 3 heads,
flash-style causal attention fully on-chip (no T x T tensor ever touches HBM),
and a partial output projection with its 192-row slice of Wproj. The host sums
the 4 partial projections per batch.

Device design notes (all matmuls float32r: 1 cycle/col at K=128, ~1e-4 rel err):
  - x arrives host-pre-transposed and k-tile-packed: [128, 6, 2048] so every
    DMA is a large 2D-contiguous transfer (descriptor-count matters: strided
    row-by-row patterns saturate the SWDGE sequencer).
  - q^T/k^T in [64, T]-per-head layout; S^T = K^T.T @ Q^T is computed
    transposed [tk, tq] so exp(S^T) feeds the P.T @ V matmul directly with no
    on-chip transposes. K^T tiles are zero-padded to K=128 (K=64 fp32r
    matmuls run 2 cycles/col) and the zero-block position selects which half
    of the shared [q0;q1] rhs tile contributes.
  - V is augmented with a ones column per head (generated by the matmul via
    the xT ones row), so the PV accumulation yields the softmax denominator
    as psum row 64 for free.
  - Causal masking: matmul columns restricted to tq >= tk-block start; the
    diagonal 128x128 sub-block gets a strictly-lower-triangular -1e30
    additive mask before exp.
  - Output is written in chunked [i, n, 128, 512] layout (contiguous DMA);
    host reassembles and reduces.
"""

import os
import sys

sys.path.insert(0, "/opt/trn_rl_repo")

import numpy as np

import concourse.bass as bass
import concourse.tile as tile
from concourse import bacc, mybir
from concourse import bass_utils
from concourse.masks import make_lower_triangular

B, T, C = 2, 2048, 768
H, D = 12, 64
N_CORES = 8
H_LOC = 3           # heads per core
DL = H_LOC * D      # 192 local head dims
TQ = 512            # tq chunk (psum bank width)
TB = 128            # tk block
NCH = T // TQ       # 4 chunks
NBL = TQ // TB      # 4 blocks per chunk
NKT = C // 128      # 6 contraction k-tiles

f32 = mybir.dt.float32
f32r = mybir.dt.float32r
EXP = mybir.ActivationFunctionType.Exp

LAST_RESULT = None  # test harness reads exec_time_ns from here


def _build_program(use_qk_bias: bool):
    from contextlib import ExitStack

    nc = bacc.Bacc("TRN2", target_bir_lowering=False, debug=False,
                   num_devices=N_CORES)

    xt_d = nc.dram_tensor("xt", [128, NKT, T], f32, kind="ExternalInput").ap()
    xt1_d = nc.dram_tensor("xt1", [1, T], f32, kind="ExternalInput").ap()
    wqk_d = nc.dram_tensor("wqk", [128, 7, 2 * DL], f32, kind="ExternalInput").ap()
    wv_d = nc.dram_tensor("wv", [128, 7, 256], f32, kind="ExternalInput").ap()
    wp_d = nc.dram_tensor("wp", [2, 128, C], f32, kind="ExternalInput").ap()
    out_d = nc.dram_tensor("outc", [NCH, C // 128, 128, TQ], f32,
                           kind="ExternalOutput").ap()

    with tile.TileContext(nc) as tc, ExitStack() as ctx:
        cpool = ctx.enter_context(tc.tile_pool(name="const", bufs=1))
        wpool = ctx.enter_context(tc.tile_pool(name="w", bufs=1))
        xpool = ctx.enter_context(tc.tile_pool(name="x", bufs=1))
        qkpool = ctx.enter_context(tc.tile_pool(name="qk", bufs=1))

        tri = cpool.tile([128, 128], f32)
        make_lower_triangular(nc, tri[:], val=-1e30, diag=False)

        # --- weights + x loads (per-k-tile DMAs so compute starts early) ---
        wqk = wpool.tile([128, 7, 2 * DL], f32r)
        for j in range(7):
            nc.sync.dma_start(wqk[:, j, :], wqk_d[:, j, :].bitcast(f32r))
        wv = wpool.tile([128, 7, 256], f32r)
        for j in range(7):
            nc.sync.dma_start(wv[:, j, :], wv_d[:, j, :].bitcast(f32r))
        xt = xpool.tile([128, NKT, T], f32r)
        for j in range(NKT):
            nc.sync.dma_start(xt[:, j, :], xt_d[:, j, :].bitcast(f32r))
        xt1 = xpool.tile([1, T], f32r)
        nc.sync.dma_start(xt1[:], xt1_d[:].bitcast(f32r))
        wp = wpool.tile([128, C], f32r)
        nc.sync.dma_start(wp[:], wp_d[0].bitcast(f32r))
        wp2 = wpool.tile([128, C], f32r)     # rows 64-127 are zeros (host pads)
        nc.sync.dma_start(wp2[:], wp_d[1].bitcast(f32r))

        # --- QKV phase ---
        # M-tiles of the packed wqk: [q0|q1], [k0|k1], [q2|k2]
        # S^T matmuls run with the contraction zero-padded to K=128. Each
        # head's K^T has the other 64 rows zeroed; the zero rows kill the
        # matching rhs rows, so one [q0;q1] rhs tile serves heads 0 and 1.
        qTA = qkpool.tile([128, T], f32r, tag="qTA")   # [q0 ; q1]
        qTC = qkpool.tile([128, T], f32r, tag="qTC")   # [q2 ; q0]
        kT0 = qkpool.tile([128, T], f32r, tag="kT0")   # [k0 ; 0]
        kT1 = qkpool.tile([128, T], f32r, tag="kT1")   # [0 ; k1]
        kT2 = qkpool.tile([128, T], f32r, tag="kT2")   # [k2 ; 0]
        v_sb = qkpool.tile([128, T // TB, 3 * (D + 1)], f32r, tag="v")

        zf = cpool.tile([64, T], f32)
        nc.vector.memset(zf[:], 0.0)
        nc.vector.tensor_copy(kT0[64:128, :], zf[:])
        nc.vector.tensor_copy(kT1[0:64, :], zf[:])
        nc.vector.tensor_copy(kT2[64:128, :], zf[:])

        prhs0 = qkpool.tile([128, T], f32r, tag="prhs0")   # heads 0,1 normalized O^T
        prhs1 = qkpool.tile([128, T], f32r, tag="prhs1")   # head 2 (rows 64+ zero)
        nc.vector.tensor_copy(prhs1[64:128, :], zf[:])

        with tc.tile_pool(name="qkv_ps", bufs=2, space="PSUM") as qkv_ps, \
             tc.tile_pool(name="v_ps", bufs=2, space="PSUM") as v_ps:
            for m in range(3):
                for t in range(NCH):
                    ps = qkv_ps.tile([128, TQ], f32)
                    for j in range(NKT):
                        nc.tensor.matmul(
                            ps[:],
                            wqk[:, j, 128 * m : 128 * (m + 1)],
                            xt[:, j, TQ * t : TQ * (t + 1)],
                            start=(j == 0),
                            stop=(j == NKT - 1 and not use_qk_bias),
                        )
                    if use_qk_bias:
                        nc.tensor.matmul(
                            ps[:],
                            wqk[0:1, 6, 128 * m : 128 * (m + 1)],
                            xt1[:, TQ * t : TQ * (t + 1)],
                            start=False, stop=True,
                        )
                    cs = slice(TQ * t, TQ * (t + 1))
                    if m == 0:
                        nc.vector.tensor_copy(qTA[:, cs], ps[:])             # q0;q1
                        nc.vector.tensor_copy(qTC[64:128, cs], ps[0:64, :])  # q0
                    elif m == 1:
                        nc.vector.tensor_copy(kT0[0:64, cs], ps[0:64, :])    # k0
                        nc.vector.tensor_copy(kT1[64:128, cs], ps[64:128, :])  # k1
                    else:
                        nc.vector.tensor_copy(qTC[0:64, cs], ps[0:64, :])    # q2
                        nc.vector.tensor_copy(kT2[0:64, cs], ps[64:128, :])  # k2
            # v in [t, d] layout; wv columns interleave [v_h | ones] per head,
            # the ones col + bias row ride on the xt ones row (k-tile 7).
            for t in range(T // TB):
                psv = v_ps.tile([128, 256], f32)
                for j in range(NKT):
                    nc.tensor.matmul(
                        psv[:],
                        xt[:, j, TB * t : TB * (t + 1)],
                        wv[:, j, :],
                        start=(j == 0), stop=False,
                    )
                nc.tensor.matmul(
                    psv[:],
                    xt1[:, TB * t : TB * (t + 1)],
                    wv[0:1, 6, :],
                    start=False, stop=True,
                )
                nc.vector.tensor_copy(v_sb[:, t, :], psv[:, 0 : 3 * (D + 1)])

        # --- attention + projection ---
        qT = [qTA, qTA, qTC]        # zero rows in kT select the head half
        kT = [kT0, kT1, kT2]

        with tc.tile_pool(name="s_ps", bufs=3, space="PSUM") as s_ps, \
             tc.tile_pool(name="po_ps", bufs=2, space="PSUM") as po_ps, \
             tc.tile_pool(name="pb_ps", bufs=2, space="PSUM") as pb_ps, \
             tc.tile_pool(name="pt_p", bufs=4) as pt_p, \
             tc.tile_pool(name="nrm", bufs=2) as nrm, \
             tc.tile_pool(name="outp", bufs=3) as outp:
            for i in range(NCH):
                nblk = NBL * (i + 1)
                for h in range(H_LOC):
                    po = po_ps.tile([D + 1, TQ], f32, tag="po")
                    for Bq in range(nblk):
                        j = Bq - NBL * i
                        c0 = 0 if j < 0 else min(TB * j, TQ - 256)
                        ps = s_ps.tile([128, TQ], f32, tag="s")
                        nc.tensor.matmul(
                            ps[:, c0:TQ],
                            kT[h][:, TB * Bq : TB * (Bq + 1)],
                            qT[h][:, TQ * i + c0 : TQ * (i + 1)],
                            start=True, stop=True,
                        )
                        if j >= 0:
                            if TB * j > c0:
                                nc.vector.tensor_scalar_add(
                                    ps[:, c0 : TB * j], ps[:, c0 : TB * j], -1e30)
                            nc.vector.tensor_add(
                                ps[:, TB * j : TB * (j + 1)],
                                ps[:, TB * j : TB * (j + 1)],
                                tri[:],
                            )
                        pt = pt_p.tile([128, TQ], f32r, tag="pt")
                        nc.scalar.activation(pt[:, c0:TQ], ps[:, c0:TQ], EXP)
                        nc.tensor.matmul(
                            po[:, c0:TQ],
                            v_sb[:, Bq, (D + 1) * h : (D + 1) * (h + 1)],
                            pt[:, c0:TQ],
                            start=(Bq == 0), stop=(Bq == nblk - 1),
                        )
                    # normalize: row D of po is the softmax denominator.
                    # Broadcast it across 64 partitions via a rank-1 matmul,
                    # take a fast approx reciprocal, multiply into O^T.
                    d_sb = nrm.tile([1, TQ], f32r, tag="d")
                    nc.vector.tensor_copy(d_sb[:], po[D : D + 1, :])
                    pb = pb_ps.tile([D, TQ], f32, tag="pb")
                    nc.tensor.matmul(pb[:], xt1[0:1, 0:D], d_sb[:], start=True, stop=True)
                    rb = nrm.tile([D, TQ], f32, tag="rb")
                    nc.vector.reciprocal_approx_fast(rb[:], pb[:])
                    dst = (prhs0[64 * h : 64 * (h + 1), TQ * i : TQ * (i + 1)]
                           if h < 2 else prhs1[0:64, TQ * i : TQ * (i + 1)])
                    nc.vector.tensor_mul(dst, po[0:D, :], rb[:])
                # projection for this chunk (wp2/prhs1 zero-padded to K=128)
                for n in range(C // 128):
                    pp = pb_ps.tile([128, TQ], f32, tag="pb")
                    nc.tensor.matmul(pp[:], wp[:, 128 * n : 128 * (n + 1)],
                                     prhs0[:, TQ * i : TQ * (i + 1)],
                                     start=True, stop=False)
                    nc.tensor.matmul(pp[:], wp2[:, 128 * n : 128 * (n + 1)],
                                     prhs1[:, TQ * i : TQ * (i + 1)],
                                     start=False, stop=True)
                    osb = outp.tile([128, TQ], f32, tag="out")
                    nc.vector.tensor_copy(osb[:], pp[:])
                    nc.sync.dma_start(out_d[i, n], osb[:])

    nc.compile()
    return nc


_PROG_CACHE = {}


def kernel(x, Wqkv, bqkv, Wproj, bproj):
    global LAST_RESULT
    x = np.asarray(x, dtype=np.float32)
    Wqkv = np.asarray(Wqkv, dtype=np.float32)
    bqkv = np.asarray(bqkv, dtype=np.float32)
    Wproj = np.asarray(Wproj, dtype=np.float32)
    bproj = np.asarray(bproj, dtype=np.float32)

    Wq, Wk, Wv = Wqkv[:, 0:C], Wqkv[:, C : 2 * C], Wqkv[:, 2 * C : 3 * C]
    bq, bk, bv = bqkv[0:C], bqkv[C : 2 * C], bqkv[2 * C : 3 * C]
    scale = 1.0 / np.sqrt(D)

    use_qk_bias = bool(np.any(bq) or np.any(bk))
    if use_qk_bias not in _PROG_CACHE:
        _PROG_CACHE[use_qk_bias] = _build_program(use_qk_bias)
    nc = _PROG_CACHE[use_qk_bias]

    in_maps = []
    for c in range(N_CORES):
        b = c // (N_CORES // B)
        g = c % (N_CORES // B)
        hs = slice(DL * g, DL * (g + 1))       # this core's head-dim rows/cols

        # x^T k-tile-packed: [128, 6, T]
        xt = np.ascontiguousarray(
            x[b].T.reshape(NKT, 128, T).transpose(1, 0, 2))
        xt1 = np.ones((1, T), np.float32)

        wq_loc = Wq[:, hs] * scale             # fold 1/sqrt(D) into q
        bq_loc = bq[hs] * scale
        wk_loc, bk_loc = Wk[:, hs], bk[hs]
        wv_loc, bv_loc = Wv[:, hs], bv[hs]

        wqk = np.zeros((C + 128, 2 * DL), np.float32)   # 7 k-tiles of 128
        wqk[0:C, 0:128] = wq_loc[:, 0:128]
        wqk[C, 0:128] = bq_loc[0:128]
        wqk[0:C, 128:256] = wk_loc[:, 0:128]
        wqk[C, 128:256] = bk_loc[0:128]
        wqk[0:C, 256:320] = wq_loc[:, 128:192]
        wqk[C, 256:320] = bq_loc[128:192]
        wqk[0:C, 320:384] = wk_loc[:, 128:192]
        wqk[C, 320:384] = bk_loc[128:192]
        wqk = np.ascontiguousarray(wqk.reshape(7, 128, 2 * DL).transpose(1, 0, 2))

        wv_pad = np.zeros((C + 128, 256), np.float32)
        for h in range(H_LOC):
            c0 = (D + 1) * h
            wv_pad[0:C, c0 : c0 + D] = wv_loc[:, D * h : D * (h + 1)]
            wv_pad[C, c0 : c0 + D] = bv_loc[D * h : D * (h + 1)]
            wv_pad[C, c0 + D] = 1.0            # ones column -> softmax denom
        wv_pad = np.ascontiguousarray(wv_pad.reshape(7, 128, 256).transpose(1, 0, 2))

        wp = np.zeros((2, 128, C), np.float32)
        wp[0] = Wproj[DL * g : DL * g + 128, :]
        wp[1, 0:64] = Wproj[DL * g + 128 : DL * (g + 1), :]

        in_maps.append({"xt": xt, "xt1": xt1, "wqk": wqk, "wv": wv_pad, "wp": wp})

    res = bass_utils.run_bass_kernel_spmd(nc, in_maps, core_ids=list(range(N_CORES)))
    LAST_RESULT = res

    out = np.zeros((B, T, C), np.float32)
    for c in range(N_CORES):
        b = c // (N_CORES // B)
        # outc [i, n, 128, 512] -> [C, T] -> [T, C]
        outT = res.results[c]["outc"].transpose(1, 2, 0, 3).reshape(C, T)
        out[b] += outT.T
    return out + bproj


if __name__ == "__main__":
    rng = np.random.default_rng(0)
    s = 1.0 / np.sqrt(C)
    ins = {
        "x": rng.standard_normal((B, T, C), dtype=np.float32),
        "Wqkv": rng.standard_normal((C, 3 * C), dtype=np.float32) * s,
        "bqkv": np.zeros(3 * C, np.float32),
        "Wproj": rng.standard_normal((C, C), dtype=np.float32) * s,
        "bproj": np.zeros(C, np.float32),
    }
    out = kernel(**ins)
    print("out", out.shape, out.dtype, float(np.abs(out).max()))
